# revision 7
# baseline (speedup 1.0000x reference)
"""AttnTopKPool Trainium2 kernel.

reference:
    w_mean = mean(w, axis=1)          # [B, S, S] -> [B, S]
    idx    = top_k(w_mean, 16)        # [B, 16]
    out    = x[b, :, idx[b]]          # [B, F, 16]

Strategy (8 NeuronCores, batch-parallel, 4 batches each):
  - host: transpose x to x_t[b, s, f] so the device gather is a contiguous
    row gather; slice w and x_t per core.
  - device per batch:
      * stream w[b] (16 MiB) in 4 MiB contiguous [128, 8192] tiles; each
        SBUF partition holds 4 full rows of w[b].
      * column sums via TensorE: ones[128,1].T @ tile chunk -> PSUM [1,512]
        accumulated over all row-tiles (top-k of sum == top-k of mean).
      * top-16 via DVE max8 / max_index / match_replace (two passes).
      * gather: per index, load it into a GpSimd register and issue a
        dynamic-offset DMA fetching that 4 KiB row of x_t[b].
        (offset-table indirect DMA is not supported on this runtime.)
  - out per core: [64, 1024] = (b_loc*16 + k, f); host reassembles to
    [B, F, K].
"""

import numpy as np

B, F, S, K = 32, 1024, 2048, 16
N_CORES = 8
B_LOC = B // N_CORES  # 4
P = 128
ROWS_PER_PART = 4          # w rows per SBUF partition in one load
LOAD_FREE = ROWS_PER_PART * S   # 8192 floats = 32 KiB per partition
LOADS_PER_B = S // (P * ROWS_PER_PART)  # 4 loads of [128, 8192] per batch
MM_N = 512                 # fp32 moving-operand max / one PSUM bank
NEG = -3.0e38              # below any column sum

_cached_nc = None

# test-only knobs (harness leaves these at defaults)
TRACE = False
_last_results = None


def _build_nc():
    from concourse import bacc, bass, mybir, tile

    f32 = mybir.dt.float32
    u32 = mybir.dt.uint32

    nc = bacc.Bacc("TRN2", target_bir_lowering=False, debug=False)

    w_d = nc.dram_tensor("w", [B_LOC, S, S], f32, kind="ExternalInput")
    xt_d = nc.dram_tensor("xt", [B_LOC, S, F], f32, kind="ExternalInput")
    out_d = nc.dram_tensor("out", [B_LOC * K, F], f32, kind="ExternalOutput")

    # [LOADS, 128, 8192]; partition p of load l holds w rows (l*512 + 4p .. +3)
    w_view = (
        w_d[:]
        .rearrange("b r s -> (b r) s")
        .rearrange("(l p four) s -> l p (four s)", p=P, four=ROWS_PER_PART)
    )

    with tile.TileContext(nc) as tc:
        with (
            tc.tile_pool(name="wpool", bufs=3) as wpool,
            tc.tile_pool(name="pspool", bufs=2, space="PSUM") as pspool,
            tc.tile_pool(name="tk", bufs=1) as tk,
        ):
            ones = tk.tile([P, 1], f32)
            nc.vector.memset(ones[:], 1.0)
            gath = tk.tile([B_LOC * K, F], f32)

            for b in range(B_LOC):
                ps = [
                    pspool.tile([1, MM_N], f32, name=f"ps{b}_{q}", tag=f"ps{q}")
                    for q in range(4)
                ]
                for l in range(LOADS_PER_B):
                    wt = wpool.tile([P, LOAD_FREE], f32, name=f"wt{b}_{l}", tag="wt")
                    nc.sync.dma_start(wt[:], w_view[b * LOADS_PER_B + l])
                    for c in range(LOAD_FREE // MM_N):  # 16 chunks
                        q = c % 4
                        nc.tensor.matmul(
                            ps[q][:],
                            ones[:],
                            wt[:, c * MM_N : (c + 1) * MM_N],
                            start=(l == 0 and c < 4),
                            stop=(l == LOADS_PER_B - 1 and c >= 12),
                        )

                sums = tk.tile([1, S], f32, name=f"sums{b}")
                for q in range(4):
                    nc.scalar.activation(
                        sums[:, q * MM_N : (q + 1) * MM_N],
                        ps[q][:],
                        mybir.ActivationFunctionType.Copy,
                    )

                gidx = tk.tile([1, K], u32, name=f"gidx{b}")
                m8a = tk.tile([1, 8], f32, name=f"m8a{b}")
                nc.vector.max(m8a[:], sums[:])
                nc.vector.max_index(gidx[:, 0:8], m8a[:], sums[:])
                nc.vector.match_replace(sums[:], m8a[:], sums[:], NEG)
                m8b = tk.tile([1, 8], f32, name=f"m8b{b}")
                nc.vector.max(m8b[:], sums[:])
                nc.vector.max_index(gidx[:, 8:16], m8b[:], sums[:])

                for k in range(K):
                    regs = nc.alloc_registers(
                        name=f"ri{b}_{k}", engines=(mybir.EngineType.Pool,)
                    )
                    reg = list(regs)[0]
                    nc.gpsimd.reg_load(reg, gidx[0:1, k : k + 1])
                    val = nc.gpsimd.snap(reg, donate=True, min_val=0, max_val=S - 1)
                    nc.gpsimd.dma_start(
                        gath[b * K + k : b * K + k + 1, :],
                        xt_d[b][bass.ds(val, 1), :],
                    )

            nc.sync.dma_start(out_d[:], gath[:])

    nc.compile()
    return nc


def _get_nc():
    global _cached_nc
    if _cached_nc is None:
        _cached_nc = _build_nc()
    return _cached_nc


def kernel(x: np.ndarray, w: np.ndarray) -> np.ndarray:
    from concourse import bass_utils

    x = np.asarray(x, dtype=np.float32)
    w = np.asarray(w, dtype=np.float32)
    x_t = np.ascontiguousarray(x.transpose(0, 2, 1))  # [B, S, F]

    nc = _get_nc()
    in_maps = [
        {
            "w": np.ascontiguousarray(w[c * B_LOC : (c + 1) * B_LOC]),
            "xt": x_t[c * B_LOC : (c + 1) * B_LOC],
        }
        for c in range(N_CORES)
    ]
    res = bass_utils.run_bass_kernel_spmd(
        nc, in_maps, list(range(N_CORES)), trace=TRACE
    )
    global _last_results
    _last_results = res
    out = np.concatenate([res.results[c]["out"] for c in range(N_CORES)], axis=0)
    # [B*K, F] -> [B, K, F] -> [B, F, K]
    return np.ascontiguousarray(out.reshape(B, K, F).transpose(0, 2, 1))
